# revision 11
# baseline (speedup 1.0000x reference)
"""Trainium2 Bass kernel for the attention-scoring module:

    q = query @ Wq.T + bq                               # (B, D)
    ref[b,d,k] = sum_e enc[k,b,e] * Wref[d,e] + bref[d]
    u[b,k] = sum_d v[d] * tanh(ref[b,d,k] + q[b,d])
    out = 10 * tanh(u)                                  # (B, K)

Data-parallel over batch: core c owns b in [32c, 32c+32).

Per-core dataflow v2 (all big tensors bf16, f32 accumulation):
  - host pre-transposes enc to (E, b*K+k); contraction dim E on SBUF
    partitions with dense DMA.
  - window (b, dc, kp): psum[d(128), 1024] += WrefT_chunk.T @ encT_chunk
    (2 ec x 2 kb matmuls, ec-outer so the stationary is reused).
  - ScalarE tanh drains each window to a bf16 tile with the per-(b,dc)
    bias folded in (free in the activation).
  - the v-weighted d-reduction: per b, 8 strip matmuls (4 PSUM
    col-groups x 2 d-chunks) with an M=32 stationary that holds v_dc in
    column b and zeros elsewhere.  Each strip adds v_dc . tanh-tile
    into row 32*jj + b of ONE persistent PSUM bank (u).  Zero columns
    add exact zeros to the other 31 rows; col-groups run concurrently.
  - final: one ScalarE tanh over the whole u bank, DVE x10, 4 strided
    output DMAs (u row 32*jj+b -> out row 4*b+jj).
"""

import os
import sys

import numpy as np

os.environ.setdefault("JAX_COMPILATION_CACHE_DIR", "/tmp/jaxcache")

for _p in ("/opt/trn_rl_repo", "/opt/pypackages"):
    if _p not in sys.path:
        sys.path.append(_p)

import ml_dtypes

E = 256
D = 256
K = 2048
B = 256
NCORES = 8
BL = B // NCORES          # 32 batch rows per core
N = BL * K                # 65536 flattened (b, k) per core
SLAB_B = 4                # b-rows per enc DMA slab
SLAB_N = SLAB_B * K       # 8192
C_CLIP = 10.0

# DVE tanh-polynomial offload: windows w with w % OFF_MOD == OFF_PHASE (in
# [OFF_LO, OFF_HI)) compute tanh of their kb0 half on VectorE as
# t = xc * p(xc^2), xc = clamp(x, +-XR), p = deg-3 poly (7th order odd).
# Fit on the real ref+q distribution; end-to-end l2 impact ~3e-3.
OFF_MOD = 6
OFF_PHASE = 1
OFF_LO = 10
OFF_HI = 116
XR = 2.6
PC = [0.978313, -0.251446, 0.043226, -0.002832]   # c1, c2, c3, c4

_compiled = None
last_exec_time_ns = None
last_results = None


def _build():
    from concourse import bacc, bass, tile

    mybir = bass.mybir
    dt = mybir.dt
    AF = mybir.ActivationFunctionType

    nc = bacc.Bacc("TRN2", target_bir_lowering=False, debug=False,
                   num_devices=NCORES)

    enc_t = nc.declare_dram_parameter("enc_t", [E, N], dt.bfloat16, isOutput=False)
    cf32_t = nc.declare_dram_parameter("cf32", [128, 578], dt.float32, isOutput=False)
    cbf16_t = nc.declare_dram_parameter("cbf16", [128, 638], dt.bfloat16, isOutput=False)
    out_p = nc.declare_dram_parameter("out", [128, 512], dt.float32, isOutput=True)

    with tile.TileContext(nc) as tc:
        with (
            tc.tile_pool(name="const", bufs=1) as constp,
            tc.tile_pool(name="enc", bufs=3) as encp,
            tc.tile_pool(name="tt", bufs=8) as ttp,
            tc.tile_pool(name="sc", bufs=8) as scp,
            tc.tile_pool(name="psum_m", bufs=3, space="PSUM") as pmp,
            tc.tile_pool(name="psum_u", bufs=1, space="PSUM") as pup,
            tc.tile_pool(name="psum_v", bufs=1, space="PSUM") as pvp,
        ):
            # ---- enc slab loading ----
            def alloc_slab(s):
                return [encp.tile([128, SLAB_N], dt.bfloat16, tag=f"enc{ec}",
                                  name=f"enc{ec}_s{s}")
                        for ec in range(2)]

            def emit_pieces(tiles, s, q0, q1, pieces=SLAB_B, dep=None):
                # ec-interleaved pieces q0..q1-1 of slab s; `dep` gates the
                # DMA issue so queued prefetches don't fair-share SDMA
                # bandwidth away from pieces that are needed right now
                w = SLAB_N // pieces
                for q in range(q0, q1):
                    for ec in range(2):
                        ins = nc.sync.dma_start(
                            tiles[ec][:, q * w:(q + 1) * w],
                            enc_t[ec * 128:(ec + 1) * 128,
                                  s * SLAB_N + q * w:s * SLAB_N + (q + 1) * w])
                        if dep is not None:
                            tile.add_dep_helper(ins.ins, dep.ins,
                                                reason="defer enc prefetch")

            def load_slab(s, pieces, dep=None):
                tiles = alloc_slab(s)
                emit_pieces(tiles, s, 0, pieces, pieces, dep=dep)
                return tiles

            # ---- constants: two packed DMAs ----
            cf32_sb = constp.tile([128, 578], dt.float32)
            cbf16_sb = constp.tile([128, 638], dt.bfloat16)
            bias_sb = constp.tile([128, 2 * BL], dt.float32)   # [:, dc*32 + b]
            warm_sb = constp.tile([128, 2], dt.float32)
            nc.gpsimd.memset(warm_sb[:], 0.0)
            # dummy tanh at t=0 so the ACT table load overlaps the DMAs
            nc.scalar.activation(warm_sb[:, 1:2], warm_sb[:, 0:1], AF.Tanh)
            nc.sync.dma_start(cf32_sb[:], cf32_t[:])
            nc.sync.dma_start(cbf16_sb[:], cbf16_t[:])

            wq_sb = cf32_sb[:, 0:512]        # [:, (ec*2+dc)*128 + d]
            query_sb = cf32_sb[:, 512:576]   # [:, ec*32 + b]
            cbias_sb = cf32_sb[:, 576:578]
            wref_sb = cbf16_sb[:, 0:512]     # [:, (ec*2+dc)*128 + d]
            vstat_sb = [cbf16_sb[:, 512:575], cbf16_sb[:, 575:638]]  # col 31 = v_dc

            t6 = constp.tile([128, 512], dt.float32)
            o6 = constp.tile([128, 512], dt.float32)

            # u accumulator: one persistent PSUM bank, row 32*jj + b
            u_ps = pup.tile([128, 512], dt.float32)
            # dedicated bank for the kb0 halves that VectorE drains
            pdve = pvp.tile([128, 512], dt.float32)

            # slab 0 staged: b0's first kp upfront, the rest fed in later
            slab0 = alloc_slab(0)
            emit_pieces(slab0, 0, 0, 1, pieces=8)

            # ---- q_rawT = (query @ Wq.T).T per d-chunk, + (bref + bq) ----
            # (borrows a psum_m slot; released before the windows wrap around)
            qps = pmp.tile([128, 1024], dt.float32, tag="psd")
            for dc in range(2):
                for ec in range(2):
                    nc.tensor.matmul(
                        qps[:, dc * BL:(dc + 1) * BL],
                        wq_sb[:, (ec * 2 + dc) * 128:(ec * 2 + dc + 1) * 128],
                        query_sb[:, ec * BL:(ec + 1) * BL],
                        start=(ec == 0), stop=(ec == 1),
                    )
                nc.vector.tensor_scalar_add(bias_sb[:, dc * BL:(dc + 1) * BL],
                                            qps[:, dc * BL:(dc + 1) * BL],
                                            cbias_sb[:, dc:dc + 1])

            def emit_quad(b, tts, dc):
                # 4 strip matmuls into the persistent u bank; the groups
                # jj run concurrently (distinct PE col-groups).  The dc=0
                # and dc=1 quads of one b accumulate into the same rows,
                # so they serialize against each other.
                for jj in range(4):
                    kp, kb = jj // 2, jj % 2
                    nc.tensor.matmul(
                        u_ps[32 * jj:32 * jj + 32, :],
                        vstat_sb[dc][:, 31 - b:63 - b],
                        tts[dc][kp][kb],
                        start=(b == 0 and dc == 0),
                        stop=(b == BL - 1 and dc == 1),
                        skip_group_check=True,
                        tile_position=(0, 32 * jj),
                    )

            def emit_poly(xps, bias_ap):
                # t = xc * p(xc^2), xc = clamp(x + bias, +-XR); returns the
                # bf16 [128, 512] tanh tile.  9 DVE ops, ~2.7us.
                OP = mybir.AluOpType
                xb = scp.tile([128, 512], dt.bfloat16, tag="sc")
                nc.vector.tensor_scalar_add(xb[:], xps, bias_ap)
                xc = scp.tile([128, 512], dt.bfloat16, tag="sc")
                nc.vector.tensor_scalar(xc[:], xb[:], XR, -XR, OP.min, OP.max)
                sq = scp.tile([128, 512], dt.bfloat16, tag="sc")
                nc.vector.tensor_mul(sq[:], xc[:], xc[:])
                h = scp.tile([128, 512], dt.bfloat16, tag="sc")
                nc.vector.tensor_scalar(h[:], sq[:], PC[3], PC[2], OP.mult, OP.add)
                h2 = scp.tile([128, 512], dt.bfloat16, tag="sc")
                nc.vector.tensor_mul(h2[:], h[:], sq[:])
                h3 = scp.tile([128, 512], dt.bfloat16, tag="sc")
                nc.vector.tensor_scalar(h3[:], h2[:], PC[1], None, OP.add)
                h4 = scp.tile([128, 512], dt.bfloat16, tag="sc")
                nc.vector.tensor_mul(h4[:], h3[:], sq[:])
                h5 = scp.tile([128, 512], dt.bfloat16, tag="sc")
                nc.vector.tensor_scalar(h5[:], h4[:], PC[0], None, OP.add)
                t = scp.tile([128, 512], dt.bfloat16, tag="tp")
                nc.vector.tensor_mul(t[:], h5[:], xc[:])
                return t

            NSLAB = N // SLAB_N
            cur_slab = slab0
            prev_mm = None
            pend = []              # deferred strip emissions: (b, tts)
            win = 0                # global window counter
            for s in range(NSLAB):                  # 8 slabs of 4 b-rows
                nxt_slab = None
                for b_in in range(SLAB_B):
                    b = SLAB_B * s + b_in
                    tts = [[None, None], [None, None]]
                    for kp in range(2):
                        for dc in range(2):
                            # startup drip-feed of slab 0 + prefetch of next
                            if s == 0 and (b_in, kp, dc) == (0, 0, 1):
                                emit_pieces(cur_slab, 0, 1, 2, pieces=8, dep=prev_mm)
                            if s == 0 and (b_in, kp, dc) == (0, 1, 0):
                                emit_pieces(cur_slab, 0, 2, 4, pieces=8, dep=prev_mm)
                            if s == 0 and (b_in, kp, dc) == (0, 1, 1):
                                emit_pieces(cur_slab, 0, 4, 8, pieces=8, dep=prev_mm)
                            pf_at = (1, 0, 0) if s == 0 else (0, 1, 0)
                            if (b_in, kp, dc) == pf_at and s + 1 < NSLAB:
                                nxt_slab = load_slab(s + 1, pieces=4, dep=prev_mm)

                            off = (win % OFF_MOD == OFF_PHASE
                                   and OFF_LO <= win < OFF_HI)
                            # strips due this window are interleaved with its
                            # matmuls: quad dc0 before, quad dc1 after the
                            # first mm, so the wref LDWEIGHTS hide under quads
                            due = None
                            if pend and pend[0][2] + 4 <= win + 1:
                                due = pend.pop(0)
                                emit_quad(due[0], due[1], 0)
                            psd = pmp.tile([128, 1024], dt.float32, tag="psd")
                            first_mm = None
                            for ec in range(2):
                                for kb in range(2):
                                    nseg = b_in * K + kp * 1024 + kb * 512
                                    dst = (pdve[:, 0:512] if (off and kb == 0)
                                           else psd[:, kb * 512:(kb + 1) * 512])
                                    ins = nc.tensor.matmul(
                                        dst,
                                        wref_sb[:, (ec * 2 + dc) * 128:(ec * 2 + dc + 1) * 128],
                                        cur_slab[ec][:, nseg:nseg + 512],
                                        start=(ec == 0), stop=(ec == 1),
                                        skip_group_check=True,
                                    )
                                    if first_mm is None:
                                        first_mm = ins
                                        if due is not None:
                                            emit_quad(due[0], due[1], 1)
                            bias_ap = bias_sb[:, dc * BL + b:dc * BL + b + 1]
                            if off:
                                t0 = emit_poly(pdve[:, 0:512], bias_ap)
                                t1 = ttp.tile([128, 512], dt.bfloat16, tag="tth")
                                nc.scalar.activation(
                                    t1[:], psd[:, 512:1024], AF.Tanh,
                                    bias=bias_ap, scale=1.0)
                                tts[dc][kp] = (t0[:], t1[:])
                            else:
                                ttile = ttp.tile([128, 1024], dt.bfloat16, tag="tt")
                                nc.scalar.activation(
                                    ttile[:], psd[:], AF.Tanh,
                                    bias=bias_ap, scale=1.0)
                                tts[dc][kp] = (ttile[:, 0:512], ttile[:, 512:1024])
                            prev_mm = first_mm
                            win += 1
                    pend.append((b, tts, win))
                cur_slab = nxt_slab
            for eb, etts, _ in pend:
                emit_quad(eb, etts, 0)
                emit_quad(eb, etts, 1)

            # ---- final: logit = 10 * tanh(u) ----
            nc.scalar.activation(t6[:], u_ps[:], AF.Tanh)
            nc.vector.tensor_scalar_mul(o6[:], t6[:], C_CLIP)
            for jj in range(4):
                # u row 32*jj + b  ->  out row 4*b + jj
                nc.sync.dma_start(out_p[jj:128:4, :], o6[32 * jj:32 * jj + 32, :])

    nc.compile()
    return nc


def _prep_inputs(encoder_output, query, Wq, bq, Wref, bref, v):
    bf16 = ml_dtypes.bfloat16
    # (K, B, E) -> (E, B, K), bf16
    enc_bf = np.asarray(encoder_output, np.float32).astype(bf16)
    encT = enc_bf.transpose(2, 1, 0)                   # (E, B, K) view

    def chunk4(w):                                     # (E, D) -> (4*128, 128)
        return np.ascontiguousarray(
            w.reshape(2, 128, 2, 128).transpose(0, 2, 1, 3).reshape(512, 128))

    def pack(w4):                                      # (4*128, X) -> (128, 4*X)
        x = w4.shape[1]
        return w4.reshape(4, 128, x).transpose(1, 0, 2).reshape(128, 4 * x)

    wref_p = pack(chunk4(np.asarray(Wref, np.float32).T))          # (128, 512)
    wq_p = pack(chunk4(np.asarray(Wq, np.float32).T))              # (128, 512)
    cbias = (np.asarray(bref, np.float32) + np.asarray(bq, np.float32))
    cbias_p = cbias.reshape(2, 128).T                               # (128, 2)
    queryT = np.ascontiguousarray(np.asarray(query, np.float32).T)  # (E, B)

    # strip stationaries: [128, 63] with v_dc in column 31, zeros elsewhere
    v_np = np.asarray(v, np.float32)
    vstat = np.zeros((128, 126), np.float32)
    vstat[:, 31] = v_np[0:128]
    vstat[:, 63 + 31] = v_np[128:256]

    cbf16 = np.concatenate([wref_p, vstat], axis=1).astype(bf16)    # (128, 638)

    in_maps = []
    for c in range(NCORES):
        enc_c = np.ascontiguousarray(encT[:, c * BL:(c + 1) * BL, :]).reshape(E, N)
        q_c = queryT[:, c * BL:(c + 1) * BL]                        # (256, 32)
        q_p = q_c.reshape(2, 128, BL).transpose(1, 0, 2).reshape(128, 2 * BL)
        cf32 = np.ascontiguousarray(np.concatenate(
            [wq_p, q_p, cbias_p], axis=1), dtype=np.float32)        # (128, 578)
        in_maps.append({
            "enc_t": enc_c,
            "cf32": cf32,
            "cbf16": cbf16,
        })
    return in_maps


def kernel(**inputs):
    global _compiled, last_exec_time_ns, last_results
    from concourse import bass_utils

    if _compiled is None:
        _compiled = _build()
    nc = _compiled

    in_maps = _prep_inputs(**inputs)
    res = bass_utils.run_bass_kernel_spmd(nc, in_maps, core_ids=list(range(NCORES)))
    last_exec_time_ns = res.exec_time_ns
    last_results = res
    # per-core (128, 512) f32 == row-major (32, 2048)
    out = np.concatenate(
        [r["out"].reshape(BL, K) for r in res.results], axis=0)
    return out


# revision 12
# speedup vs baseline: 1.0378x; 1.0378x over previous
"""Trainium2 Bass kernel for the attention-scoring module:

    q = query @ Wq.T + bq                               # (B, D)
    ref[b,d,k] = sum_e enc[k,b,e] * Wref[d,e] + bref[d]
    u[b,k] = sum_d v[d] * tanh(ref[b,d,k] + q[b,d])
    out = 10 * tanh(u)                                  # (B, K)

Data-parallel over batch: core c owns b in [32c, 32c+32).

Per-core dataflow v2 (all big tensors bf16, f32 accumulation):
  - host pre-transposes enc to (E, b*K+k); contraction dim E on SBUF
    partitions with dense DMA.
  - window (b, dc, kp): psum[d(128), 1024] += WrefT_chunk.T @ encT_chunk
    (2 ec x 2 kb matmuls, ec-outer so the stationary is reused).
  - ScalarE tanh drains each window to a bf16 tile with the per-(b,dc)
    bias folded in (free in the activation).
  - the v-weighted d-reduction: per b, 8 strip matmuls (4 PSUM
    col-groups x 2 d-chunks) with an M=32 stationary that holds v_dc in
    column b and zeros elsewhere.  Each strip adds v_dc . tanh-tile
    into row 32*jj + b of ONE persistent PSUM bank (u).  Zero columns
    add exact zeros to the other 31 rows; col-groups run concurrently.
  - final: one ScalarE tanh over the whole u bank, DVE x10, 4 strided
    output DMAs (u row 32*jj+b -> out row 4*b+jj).
"""

import os
import sys

import numpy as np

os.environ.setdefault("JAX_COMPILATION_CACHE_DIR", "/tmp/jaxcache")

for _p in ("/opt/trn_rl_repo", "/opt/pypackages"):
    if _p not in sys.path:
        sys.path.append(_p)

import ml_dtypes

E = 256
D = 256
K = 2048
B = 256
NCORES = 8
BL = B // NCORES          # 32 batch rows per core
N = BL * K                # 65536 flattened (b, k) per core
SLAB_B = 4                # b-rows per enc DMA slab
SLAB_N = SLAB_B * K       # 8192
C_CLIP = 10.0

# DVE tanh-polynomial offload: windows w with w % OFF_MOD == OFF_PHASE (in
# [OFF_LO, OFF_HI)) compute tanh of their kb0 half on VectorE as
# t = xc * p(xc^2), xc = clamp(x, +-XR), p = deg-3 poly (7th order odd).
# Fit on the real ref+q distribution; end-to-end l2 impact ~3e-3.
OFF_MOD = 6
OFF_PHASE = 1
OFF_LO = 10
OFF_HI = 0
XR = 2.6
PC = [0.978313, -0.251446, 0.043226, -0.002832]   # c1, c2, c3, c4

_compiled = None
last_exec_time_ns = None
last_results = None


def _build():
    from concourse import bacc, bass, tile

    mybir = bass.mybir
    dt = mybir.dt
    AF = mybir.ActivationFunctionType

    nc = bacc.Bacc("TRN2", target_bir_lowering=False, debug=False,
                   num_devices=NCORES)

    enc_t = nc.declare_dram_parameter("enc_t", [E, N], dt.bfloat16, isOutput=False)
    cf32_t = nc.declare_dram_parameter("cf32", [128, 578], dt.float32, isOutput=False)
    cbf16_t = nc.declare_dram_parameter("cbf16", [128, 638], dt.bfloat16, isOutput=False)
    out_p = nc.declare_dram_parameter("out", [128, 512], dt.float32, isOutput=True)

    with tile.TileContext(nc) as tc:
        with (
            tc.tile_pool(name="const", bufs=1) as constp,
            tc.tile_pool(name="enc", bufs=3) as encp,
            tc.tile_pool(name="tt", bufs=8) as ttp,
            tc.tile_pool(name="sc", bufs=8) as scp,
            tc.tile_pool(name="psum_m", bufs=3, space="PSUM") as pmp,
            tc.tile_pool(name="psum_u", bufs=1, space="PSUM") as pup,
            tc.tile_pool(name="psum_v", bufs=1, space="PSUM") as pvp,
        ):
            # ---- enc slab loading ----
            def alloc_slab(s):
                return [encp.tile([128, SLAB_N], dt.bfloat16, tag=f"enc{ec}",
                                  name=f"enc{ec}_s{s}")
                        for ec in range(2)]

            def emit_pieces(tiles, s, q0, q1, pieces=SLAB_B, dep=None):
                # ec-interleaved pieces q0..q1-1 of slab s; `dep` gates the
                # DMA issue so queued prefetches don't fair-share SDMA
                # bandwidth away from pieces that are needed right now
                w = SLAB_N // pieces
                for q in range(q0, q1):
                    for ec in range(2):
                        ins = nc.sync.dma_start(
                            tiles[ec][:, q * w:(q + 1) * w],
                            enc_t[ec * 128:(ec + 1) * 128,
                                  s * SLAB_N + q * w:s * SLAB_N + (q + 1) * w])
                        if dep is not None:
                            tile.add_dep_helper(ins.ins, dep.ins,
                                                reason="defer enc prefetch")

            def load_slab(s, pieces, dep=None):
                tiles = alloc_slab(s)
                emit_pieces(tiles, s, 0, pieces, pieces, dep=dep)
                return tiles

            # ---- constants: two packed DMAs ----
            cf32_sb = constp.tile([128, 578], dt.float32)
            cbf16_sb = constp.tile([128, 638], dt.bfloat16)
            bias_sb = constp.tile([128, 2 * BL], dt.float32)   # [:, dc*32 + b]
            warm_sb = constp.tile([128, 2], dt.float32)
            nc.gpsimd.memset(warm_sb[:], 0.0)
            # dummy tanh at t=0 so the ACT table load overlaps the DMAs
            nc.scalar.activation(warm_sb[:, 1:2], warm_sb[:, 0:1], AF.Tanh)
            nc.sync.dma_start(cf32_sb[:], cf32_t[:])
            nc.sync.dma_start(cbf16_sb[:], cbf16_t[:])

            wq_sb = cf32_sb[:, 0:512]        # [:, (ec*2+dc)*128 + d]
            query_sb = cf32_sb[:, 512:576]   # [:, ec*32 + b]
            cbias_sb = cf32_sb[:, 576:578]
            wref_sb = cbf16_sb[:, 0:512]     # [:, (ec*2+dc)*128 + d]
            vstat_sb = [cbf16_sb[:, 512:575], cbf16_sb[:, 575:638]]  # col 31 = v_dc

            t6 = constp.tile([128, 512], dt.float32)
            o6 = constp.tile([128, 512], dt.float32)

            # u accumulator: one persistent PSUM bank, row 32*jj + b
            u_ps = pup.tile([128, 512], dt.float32)
            # dedicated bank for the kb0 halves that VectorE drains
            pdve = pvp.tile([128, 512], dt.float32)

            # slab 0 staged: b0's first kp upfront, the rest fed in later
            slab0 = alloc_slab(0)
            emit_pieces(slab0, 0, 0, 1, pieces=8)

            # ---- q_rawT = (query @ Wq.T).T per d-chunk, + (bref + bq) ----
            # (borrows a psum_m slot; released before the windows wrap around)
            qps = pmp.tile([128, 1024], dt.float32, tag="psd")
            for dc in range(2):
                for ec in range(2):
                    nc.tensor.matmul(
                        qps[:, dc * BL:(dc + 1) * BL],
                        wq_sb[:, (ec * 2 + dc) * 128:(ec * 2 + dc + 1) * 128],
                        query_sb[:, ec * BL:(ec + 1) * BL],
                        start=(ec == 0), stop=(ec == 1),
                    )
                nc.vector.tensor_scalar_add(bias_sb[:, dc * BL:(dc + 1) * BL],
                                            qps[:, dc * BL:(dc + 1) * BL],
                                            cbias_sb[:, dc:dc + 1])

            def emit_quad(b, tts, dc):
                # 4 strip matmuls into the persistent u bank; the groups
                # jj run concurrently (distinct PE col-groups).  The dc=0
                # and dc=1 quads of one b accumulate into the same rows,
                # so they serialize against each other.
                for jj in range(4):
                    kp, kb = jj // 2, jj % 2
                    nc.tensor.matmul(
                        u_ps[32 * jj:32 * jj + 32, :],
                        vstat_sb[dc][:, 31 - b:63 - b],
                        tts[dc][kp][kb],
                        start=(b == 0 and dc == 0),
                        stop=(b == BL - 1 and dc == 1),
                        skip_group_check=True,
                        tile_position=(0, 32 * jj),
                    )

            def emit_poly(xps, bias_ap):
                # t = xc * p(xc^2), xc = clamp(x + bias, +-XR); returns the
                # bf16 [128, 512] tanh tile.  9 DVE ops, ~2.7us.
                OP = mybir.AluOpType
                xb = scp.tile([128, 512], dt.bfloat16, tag="sc")
                nc.vector.tensor_scalar_add(xb[:], xps, bias_ap)
                xc = scp.tile([128, 512], dt.bfloat16, tag="sc")
                nc.vector.tensor_scalar(xc[:], xb[:], XR, -XR, OP.min, OP.max)
                sq = scp.tile([128, 512], dt.bfloat16, tag="sc")
                nc.vector.tensor_mul(sq[:], xc[:], xc[:])
                h = scp.tile([128, 512], dt.bfloat16, tag="sc")
                nc.vector.tensor_scalar(h[:], sq[:], PC[3], PC[2], OP.mult, OP.add)
                h2 = scp.tile([128, 512], dt.bfloat16, tag="sc")
                nc.vector.tensor_mul(h2[:], h[:], sq[:])
                h3 = scp.tile([128, 512], dt.bfloat16, tag="sc")
                nc.vector.tensor_scalar(h3[:], h2[:], PC[1], None, OP.add)
                h4 = scp.tile([128, 512], dt.bfloat16, tag="sc")
                nc.vector.tensor_mul(h4[:], h3[:], sq[:])
                h5 = scp.tile([128, 512], dt.bfloat16, tag="sc")
                nc.vector.tensor_scalar(h5[:], h4[:], PC[0], None, OP.add)
                t = scp.tile([128, 512], dt.bfloat16, tag="tp")
                nc.vector.tensor_mul(t[:], h5[:], xc[:])
                return t

            NSLAB = N // SLAB_N
            cur_slab = slab0
            prev_mm = None
            pend = []              # deferred strip emissions: (b, tts)
            win = 0                # global window counter
            for s in range(NSLAB):                  # 8 slabs of 4 b-rows
                nxt_slab = None
                for b_in in range(SLAB_B):
                    b = SLAB_B * s + b_in
                    tts = [[None, None], [None, None]]
                    for kp in range(2):
                        for dc in range(2):
                            # startup drip-feed of slab 0 + prefetch of next
                            if s == 0 and (b_in, kp, dc) == (0, 0, 1):
                                emit_pieces(cur_slab, 0, 1, 2, pieces=8, dep=prev_mm)
                            if s == 0 and (b_in, kp, dc) == (0, 1, 0):
                                emit_pieces(cur_slab, 0, 2, 4, pieces=8, dep=prev_mm)
                            if s == 0 and (b_in, kp, dc) == (0, 1, 1):
                                emit_pieces(cur_slab, 0, 4, 8, pieces=8, dep=prev_mm)
                            pf_at = (1, 0, 0) if s == 0 else (0, 1, 0)
                            if (b_in, kp, dc) == pf_at and s + 1 < NSLAB:
                                nxt_slab = load_slab(s + 1, pieces=4, dep=prev_mm)

                            off = (win % OFF_MOD == OFF_PHASE
                                   and OFF_LO <= win < OFF_HI)
                            # strips due this window are interleaved with its
                            # matmuls: quad dc0 before, quad dc1 after the
                            # first mm, so the wref LDWEIGHTS hide under quads
                            due = None
                            if pend and pend[0][2] + 4 <= win + 1:
                                due = pend.pop(0)
                                emit_quad(due[0], due[1], 0)
                            psd = pmp.tile([128, 1024], dt.float32, tag="psd")
                            first_mm = None
                            for ec in range(2):
                                for kb in range(2):
                                    nseg = b_in * K + kp * 1024 + kb * 512
                                    dst = (pdve[:, 0:512] if (off and kb == 0)
                                           else psd[:, kb * 512:(kb + 1) * 512])
                                    ins = nc.tensor.matmul(
                                        dst,
                                        wref_sb[:, (ec * 2 + dc) * 128:(ec * 2 + dc + 1) * 128],
                                        cur_slab[ec][:, nseg:nseg + 512],
                                        start=(ec == 0), stop=(ec == 1),
                                        skip_group_check=True,
                                    )
                                    if first_mm is None:
                                        first_mm = ins
                                        if due is not None:
                                            emit_quad(due[0], due[1], 1)
                            bias_ap = bias_sb[:, dc * BL + b:dc * BL + b + 1]
                            if off:
                                t0 = emit_poly(pdve[:, 0:512], bias_ap)
                                t1 = ttp.tile([128, 512], dt.bfloat16, tag="tth")
                                nc.scalar.activation(
                                    t1[:], psd[:, 512:1024], AF.Tanh,
                                    bias=bias_ap, scale=1.0)
                                tts[dc][kp] = (t0[:], t1[:])
                            else:
                                ttile = ttp.tile([128, 1024], dt.bfloat16, tag="tt")
                                nc.scalar.activation(
                                    ttile[:], psd[:], AF.Tanh,
                                    bias=bias_ap, scale=1.0)
                                tts[dc][kp] = (ttile[:, 0:512], ttile[:, 512:1024])
                            prev_mm = first_mm
                            win += 1
                    pend.append((b, tts, win))
                cur_slab = nxt_slab
            for eb, etts, _ in pend:
                emit_quad(eb, etts, 0)
                emit_quad(eb, etts, 1)

            # ---- final: logit = 10 * tanh(u) ----
            nc.scalar.activation(t6[:], u_ps[:], AF.Tanh)
            nc.vector.tensor_scalar_mul(o6[:], t6[:], C_CLIP)
            for jj in range(4):
                # u row 32*jj + b  ->  out row 4*b + jj
                nc.sync.dma_start(out_p[jj:128:4, :], o6[32 * jj:32 * jj + 32, :])

    nc.compile()
    return nc


def _prep_inputs(encoder_output, query, Wq, bq, Wref, bref, v):
    bf16 = ml_dtypes.bfloat16
    # (K, B, E) -> (E, B, K), bf16
    enc_bf = np.asarray(encoder_output, np.float32).astype(bf16)
    encT = enc_bf.transpose(2, 1, 0)                   # (E, B, K) view

    def chunk4(w):                                     # (E, D) -> (4*128, 128)
        return np.ascontiguousarray(
            w.reshape(2, 128, 2, 128).transpose(0, 2, 1, 3).reshape(512, 128))

    def pack(w4):                                      # (4*128, X) -> (128, 4*X)
        x = w4.shape[1]
        return w4.reshape(4, 128, x).transpose(1, 0, 2).reshape(128, 4 * x)

    wref_p = pack(chunk4(np.asarray(Wref, np.float32).T))          # (128, 512)
    wq_p = pack(chunk4(np.asarray(Wq, np.float32).T))              # (128, 512)
    cbias = (np.asarray(bref, np.float32) + np.asarray(bq, np.float32))
    cbias_p = cbias.reshape(2, 128).T                               # (128, 2)
    queryT = np.ascontiguousarray(np.asarray(query, np.float32).T)  # (E, B)

    # strip stationaries: [128, 63] with v_dc in column 31, zeros elsewhere
    v_np = np.asarray(v, np.float32)
    vstat = np.zeros((128, 126), np.float32)
    vstat[:, 31] = v_np[0:128]
    vstat[:, 63 + 31] = v_np[128:256]

    cbf16 = np.concatenate([wref_p, vstat], axis=1).astype(bf16)    # (128, 638)

    in_maps = []
    for c in range(NCORES):
        enc_c = np.ascontiguousarray(encT[:, c * BL:(c + 1) * BL, :]).reshape(E, N)
        q_c = queryT[:, c * BL:(c + 1) * BL]                        # (256, 32)
        q_p = q_c.reshape(2, 128, BL).transpose(1, 0, 2).reshape(128, 2 * BL)
        cf32 = np.ascontiguousarray(np.concatenate(
            [wq_p, q_p, cbias_p], axis=1), dtype=np.float32)        # (128, 578)
        in_maps.append({
            "enc_t": enc_c,
            "cf32": cf32,
            "cbf16": cbf16,
        })
    return in_maps


def kernel(**inputs):
    global _compiled, last_exec_time_ns, last_results
    from concourse import bass_utils

    if _compiled is None:
        _compiled = _build()
    nc = _compiled

    in_maps = _prep_inputs(**inputs)
    res = bass_utils.run_bass_kernel_spmd(nc, in_maps, core_ids=list(range(NCORES)))
    last_exec_time_ns = res.exec_time_ns
    last_results = res
    # per-core (128, 512) f32 == row-major (32, 2048)
    out = np.concatenate(
        [r["out"].reshape(BL, K) for r in res.results], axis=0)
    return out
